# revision 1
# baseline (speedup 1.0000x reference)
"""Trainium2 Bass kernel for nn_Lowpass: EMA recurrence over time.

level_t = (1-s) * x_t + s * level_{t-1},  s = sigmoid(smoothing_var)

Strategy:
  - Data-parallel over batch: 16 batches -> 8 cores x 2 batches.
  - Time is processed in blocks of L=128 (the SBUF partition dim).
    Within a block, the whole recurrence is a lower-triangular matmul:
        y[j] = sum_{i<=j} A[j,i] x[i] + s^(j+1) * c        (c = carry)
        A[j,i] = (1-s) * s^(j-i)
    The carry term folds into the matmul as a row-0 fixup, since
    A @ (x + e_0 * (s/(1-s)) * c) = A @ x + p * c with p[j] = s^(j+1).
  - TensorE does the 128x128 prefix matmul (float32r: full-rate fp32),
    DVE does the tiny [1,U] carry fixups, ScalarE copies PSUM->SBUF,
    DMA streams x in and y out. Memory-bound by design.
"""

import os
import sys
import functools

sys.path.insert(0, "/opt/trn_rl_repo")
os.environ.setdefault("MYCRO_LOCAL_CACHE", "1")

import numpy as np

B, T, U = 16, 2048, 1024
NCORES = 8
BL = B // NCORES          # batches per core
L = 128                   # time block == partition dim
NBLK = T // L
H = 512                   # matmul moving-free max for fp32
NH = U // H
USE_F32R = os.environ.get("LOWPASS_F32R", "0") == "1"
GRP = int(os.environ.get("LOWPASS_GRP", "4"))


@functools.lru_cache(maxsize=8)
def _build(use_f32r: bool, grp: int = 4, sf: float = 0.0):
    """grp = time blocks per DMA group (grp*0.5MB per transfer).

    use_f32r: run the prefix matmul in float32r (full-rate fp32 path on
    the PE).  The x/A SBUF tiles are *declared* float32r so every
    producer (DMA, DVE fixup) emits that dtype, which the BIR verifier
    requires for FP32r matmul inputs.
    """
    import concourse.tile as tile
    from concourse import bacc, mybir

    nc = bacc.Bacc("TRN2", target_bir_lowering=False, debug=False)
    f32 = mybir.dt.float32
    mmdt = mybir.dt.float32r if use_f32r else f32
    x = nc.dram_tensor("x", [BL, T, U], f32, kind="ExternalInput").ap()
    at = nc.dram_tensor("at", [L, L], f32, kind="ExternalInput").ap()
    pc = nc.dram_tensor("pc", [1, L], f32, kind="ExternalInput").ap()
    c0 = nc.dram_tensor("c0", [1, U], f32, kind="ExternalInput").ap()
    y = nc.dram_tensor("y", [BL, T, U], f32, kind="ExternalOutput").ap()

    NG = NBLK // grp  # DMA groups per batch
    # dram view: [b, g, p, n, u] so one group DMA fills SBUF [128, grp*U]
    xr = x.rearrange("b (g n p) u -> b g p n u", n=grp, p=L)
    yr = y.rearrange("b (g n p) u -> b g p n u", n=grp, p=L)

    with tile.TileContext(nc) as tc:
        with (
            tc.tile_pool(name="const", bufs=1) as constp,
            tc.tile_pool(name="xin", bufs=3) as xinp,
            tc.tile_pool(name="yout", bufs=3) as youtp,
            tc.tile_pool(name="ypsum", bufs=8, space="PSUM") as ypp,
        ):
            att = constp.tile([L, L], mmdt)
            nc.sync.dma_start(att[:, :], at.bitcast(mmdt))
            pct = constp.tile([1, L], mmdt)
            nc.sync.dma_start(pct[:, :], pc.bitcast(mmdt))
            c0t = constp.tile([1, U], mmdt)
            nc.sync.dma_start(c0t[:, :], c0.bitcast(mmdt))

            # 4 independent carry chains: (batch, u-half).  Per block,
            # two structures depending on matmul dtype:
            #   f32r: yp = Arot @ x_blk (mm1) then yp += p_rot (x) carry
            #         (mm2, PSUM accumulate; carry = prev block's SBUF row 0)
            #   f32:  carry fixup on DVE (x'[0,:] += s/(1-s) * carry read
            #         straight from prev PSUM partition 0), then one matmul.
            #         Avoids mm2, which at fp32's 4 cycles/row would double
            #         the PE load.  (y rows are rotated by +1, so the
            #         block's LAST time step sits at partition/row 0.)
            mult = mybir.AluOpType.mult
            add = mybir.AluOpType.add
            sfv = float(sf)
            prevc = [[c0t[0:1, h * H : (h + 1) * H] for h in range(NH)]
                     for _ in range(BL)]
            prevp = [[None] * NH for _ in range(BL)]
            for g in range(NG):
                for b in range(BL):
                    xt = xinp.tile([L, grp * U], mmdt)
                    xt3 = xt[:, :].rearrange("p (n u) -> p n u", n=grp)
                    nc.sync.dma_start(xt3, xr[b, g].bitcast(mmdt))
                    yt = youtp.tile([L, grp * U], mmdt)
                    for n in range(grp):
                        for h in range(NH):
                            xb = xt[:, n * U + h * H : n * U + (h + 1) * H]
                            yp = ypp.tile([L, H], f32)
                            if use_f32r:
                                nc.tensor.matmul(
                                    yp[:, :], lhsT=att[:, :], rhs=xb[:, :],
                                    start=True, stop=False,
                                )
                                nc.tensor.matmul(
                                    yp[:, :], lhsT=pct[:, :], rhs=prevc[b][h],
                                    start=False, stop=True,
                                )
                            else:
                                cin = (
                                    c0t[0:1, h * H : (h + 1) * H]
                                    if (g == 0 and n == 0)
                                    else prevp[b][h][0:1, :]
                                )
                                nc.vector.scalar_tensor_tensor(
                                    out=xb[0:1, :], in0=cin, scalar=sfv,
                                    in1=xb[0:1, :], op0=mult, op1=add,
                                )
                                nc.tensor.matmul(
                                    yp[:, :], lhsT=att[:, :], rhs=xb[:, :],
                                    start=True, stop=True,
                                )
                            nc.scalar.activation(
                                yt[:, n * U + h * H : n * U + (h + 1) * H],
                                yp[:, :], mybir.ActivationFunctionType.Copy,
                            )
                            prevc[b][h] = yt[
                                0:1, n * U + h * H : n * U + (h + 1) * H
                            ]
                            prevp[b][h] = yp
                    # un-rotate: SBUF rows 1..127 -> DRAM rows 0..126,
                    # SBUF row 0 -> DRAM row 127 (two positive-stride DMAs)
                    yt3a = (yt[1:L, :].bitcast(f32)
                            .rearrange("p (n u) -> p n u", n=grp))
                    nc.gpsimd.dma_start(yr[b, g][0 : L - 1], yt3a)
                    yt3b = (yt[0:1, :].bitcast(f32)
                            .rearrange("p (n u) -> p n u", n=grp))
                    nc.gpsimd.dma_start(yr[b, g][L - 1 : L], yt3b)
    nc.compile()
    return nc


def _host_params(smoothing_var: np.ndarray, dtype=np.float64):
    """s (fp32 scalar, as reference computes it), A^T matrix, sf."""
    sm = smoothing_var.astype(np.float32).reshape(-1)
    s32 = (1.0 / (1.0 + np.exp(-sm.astype(np.float64)))).astype(np.float32)
    return s32


def _host_mats(s32_scalar):
    """Stationary matrix (row-reversed A, transposed for lhsT) and sf."""
    s = np.float64(s32_scalar)
    j = np.arange(L)[:, None]
    i = np.arange(L)[None, :]
    A = np.where(j >= i, (1.0 - s) * s ** (j - i), 0.0)
    Arot = np.roll(A, 1, axis=0)  # PSUM row m = y[(m-1) % 128]; row 0 = carry
    AT = np.ascontiguousarray(Arot.T.astype(np.float32))
    m = np.arange(L)
    pcol = (s ** (((m - 1) % L) + 1)).astype(np.float32).reshape(1, L)
    return AT, np.ascontiguousarray(pcol), float(s / (1.0 - s))


def kernel(inputs: np.ndarray, level_var: np.ndarray, smoothing_var: np.ndarray):
    from concourse import bass_utils

    x = np.ascontiguousarray(inputs, dtype=np.float32)
    assert x.shape == (B, T, U), x.shape
    s32 = _host_params(smoothing_var)
    if not np.all(s32 == s32[0]):
        # general per-unit s: fall back to exact numpy recurrence
        return _kernel_numpy(x, level_var, s32)
    AT, pcol, sf = _host_mats(s32[0])
    c0 = np.ascontiguousarray(level_var.astype(np.float32).reshape(1, U))

    nc = _build(USE_F32R, GRP, sf)
    in_maps = [
        {"x": np.ascontiguousarray(x[c * BL : (c + 1) * BL]), "at": AT,
         "pc": pcol, "c0": c0}
        for c in range(NCORES)
    ]
    res = bass_utils.run_bass_kernel_spmd(nc, in_maps, core_ids=list(range(NCORES)))
    out = np.concatenate([res.results[c]["y"] for c in range(NCORES)], axis=0)
    return out


def _kernel_numpy(x, level_var, s32):
    out = np.empty_like(x)
    c = np.broadcast_to(level_var.reshape(1, U), (x.shape[0], U)).astype(np.float32)
    one_minus = (1.0 - s32).astype(np.float32)
    for t in range(x.shape[1]):
        c = one_minus * x[:, t] + s32 * c
        out[:, t] = c
    return out


if __name__ == "__main__":
    rng = np.random.default_rng(0)
    xs = rng.standard_normal((B, T, U)).astype(np.float32)
    e = np.exp(-0.001 / 0.1)
    sm = np.full((1, U), np.log(e / (1 - e)), np.float32)
    lv = np.zeros((1, U), np.float32)
    o = kernel(xs, lv, sm)
    print("out", o.shape, o.dtype, float(np.abs(o).max()))



# revision 2
# speedup vs baseline: 2.7790x; 2.7790x over previous
"""Trainium2 Bass kernel for nn_Lowpass: EMA recurrence over time.

y[b,t,u] = (1-s_u) x[b,t,u] + s_u y[b,t-1,u],   s = sigmoid(smoothing_var)

Strategy (scan formulation):
  - The DVE/Pool ISA op TensorTensorScanArith computes exactly this
    first-order recurrence along the free dimension with an fp32 internal
    state:  state = data0[:,t] * state + data1[:,t].
  - Layout: host transposes x to [batch, unit, time] so time is the free
    (contiguous) dim; partitions carry 128 units.  Per core (2 batches):
    16 tiles of [128 units, 2048 time], one scan op per tile - no carry
    chain, no matmuls, no PSUM.
  - data1 = (1-s)*x, pre-scaled on host (exact f32), stored fp16.
  - data0 = s must be fp16 (any fp32 operand would halve DVE throughput
    and fp16's grid near 0.99 is ~4.9e-4 coarse, a 1.9% time-constant
    error).  Instead data0 is a sigma-delta dither between the two fp16
    grid neighbours of s chosen so every windowed product of data0 values
    tracks s^k to ~1e-3 - the filter shape error drops to ~0.1%.
  - Work split: scans on DVE (2194ns/tile) and Pool (1707ns/tile), 7/9.
    DMA across 4 queues (SP, Activation, DVE-HWDGE, Pool-SWDGE); each
    queue's transfers serialize but distinct queues overlap fully.
"""

import os
import sys
import functools

sys.path.insert(0, "/opt/trn_rl_repo")
os.environ.setdefault("MYCRO_LOCAL_CACHE", "1")

import numpy as np

B, T, U = 16, 2048, 1024
NCORES = 8
BL = B // NCORES            # batches per core
NTILES = BL * U // 128      # scan tiles per core
F = T                       # scan free length

USE_DVE_Q = os.environ.get("LOWPASS_DVEQ", "1") == "1"
N_DVE = int(os.environ.get("LOWPASS_NDVE", "7"))  # tiles scanned on DVE


@functools.lru_cache(maxsize=8)
def _build(uniform_s: bool, use_init: bool, dve_q: bool, n_dve: int):
    import concourse.tile as tile
    from concourse import bacc, mybir

    nc = bacc.Bacc("TRN2", target_bir_lowering=False, debug=False)
    if dve_q:
        nc.sync.bass.hwdge_engines.add(mybir.EngineType.DVE)
    f32 = mybir.dt.float32
    f16 = mybir.dt.float16
    mult = mybir.AluOpType.mult
    add = mybir.AluOpType.add

    x = nc.dram_tensor("x", [NTILES, 128, F], f16, kind="ExternalInput").ap()
    sp_shape = [1 if uniform_s else NTILES, 128, F]
    sp = nc.dram_tensor("sp", sp_shape, f16, kind="ExternalInput").ap()
    c0 = nc.dram_tensor("c0", [NTILES, 128, 1], f32, kind="ExternalInput").ap()
    y = nc.dram_tensor("y", [NTILES, 128, F], f16, kind="ExternalOutput").ap()

    queues = [nc.sync, nc.scalar, nc.gpsimd] + ([nc.vector] if dve_q else [])
    nq = len(queues)

    # interleave DVE/Pool scan tiles so both engines start early
    engs = []
    a = b = 0
    for i in range(NTILES):
        if a * (NTILES - n_dve) <= b * n_dve and a < n_dve:
            engs.append("vector"); a += 1
        else:
            engs.append("gpsimd"); b += 1

    with tile.TileContext(nc) as tc:
        with (
            tc.tile_pool(name="const", bufs=1) as constp,
            tc.tile_pool(name="xin", bufs=6) as xinp,
            tc.tile_pool(name="yout", bufs=6) as youtp,
        ):
            if uniform_s:
                st = constp.tile([128, F], f16)
                nc.sync.dma_start(st[:, :], sp[0])
                stiles = [st] * NTILES
            else:
                stiles = []
                for i in range(NTILES):
                    sti = constp.tile([128, F], f16)
                    queues[i % nq].dma_start(sti[:, :], sp[i])
                    stiles.append(sti)
            if use_init:
                c0t = constp.tile([128, NTILES], f32)
                nc.scalar.dma_start(
                    c0t[:, :], c0.rearrange("n p o -> p (n o)")
                )

            for i in range(NTILES):
                xt = xinp.tile([128, F], f16)
                queues[(2 * i) % nq].dma_start(xt[:, :], x[i])
                yt = youtp.tile([128, F], f16)
                init = c0t[:, i : i + 1] if use_init else 0.0
                getattr(nc, engs[i]).tensor_tensor_scan(
                    out=yt[:, :], data0=stiles[i][:, :], data1=xt[:, :],
                    initial=init, op0=mult, op1=add,
                )
                queues[(2 * i + 1) % nq].dma_start(y[i], yt[:, :])
    nc.compile()
    return nc


def _fp16_neighbors(s64):
    """The two fp16 grid values bracketing s (log-space)."""
    a = np.float16(s64)
    fa = float(np.float64(a))
    if fa == s64:
        return a, a
    if fa < s64:
        b = np.nextafter(a, np.float16(np.inf))
    else:
        b = np.nextafter(a, np.float16(-np.inf))
    return a, b


def _dither_pattern(s64, n):
    """fp16 sequence whose running products track s64^k to ~one grid step."""
    va, vb = _fp16_neighbors(s64)
    if va == vb:
        return np.full(n, va, np.float16)
    la = np.log(np.float64(va))
    lb = np.log(np.float64(vb))
    tgt = np.log(np.float64(s64))
    out = np.empty(n, np.float16)
    cum = 0.0
    for t in range(n):
        ea = cum + la - tgt
        eb = cum + lb - tgt
        if abs(ea) <= abs(eb):
            out[t] = va
            cum = ea
        else:
            out[t] = vb
            cum = eb
    return out


def _prep_host(inputs, level_var, smoothing_var):
    x = np.asarray(inputs, np.float32)
    assert x.shape == (B, T, U), x.shape
    sm = np.asarray(smoothing_var, np.float32).reshape(-1)
    assert sm.shape == (U,), sm.shape
    s64 = 1.0 / (1.0 + np.exp(-sm.astype(np.float64)))
    s32 = s64.astype(np.float32)

    # data1 = (1-s)*x, transposed to [B, U, T], fp16
    xs = x * (np.float32(1.0) - s32)[None, None, :]
    xsT = np.ascontiguousarray(np.swapaxes(xs, 1, 2)).astype(np.float16)

    uniform = bool(np.all(s64 == s64[0]))
    if uniform:
        pat = _dither_pattern(float(s64[0]), F)
        spat = np.ascontiguousarray(
            np.broadcast_to(pat[None, None, :], (1, 128, F))
        )
    else:
        spat = np.empty((U, F), np.float16)
        seen = {}
        for u in range(U):
            k = float(s64[u])
            if k not in seen:
                seen[k] = _dither_pattern(k, F)
            spat[u] = seen[k]

    lv = np.asarray(level_var, np.float32).reshape(-1)
    use_init = bool(np.any(lv != 0.0))
    return xsT, spat, s32, lv, uniform, use_init


def kernel(inputs: np.ndarray, level_var: np.ndarray, smoothing_var: np.ndarray):
    from concourse import bass_utils

    xsT, spat, s32, lv, uniform, use_init = _prep_host(
        inputs, level_var, smoothing_var
    )
    nc = _build(uniform, use_init, USE_DVE_Q, N_DVE)

    c0_full = np.broadcast_to(lv[None, :], (BL, U)).reshape(NTILES, 128, 1)
    in_maps = []
    for c in range(NCORES):
        xc = xsT[c * BL : (c + 1) * BL].reshape(NTILES, 128, F)
        m = {"x": np.ascontiguousarray(xc)}
        if uniform:
            m["sp"] = spat
        else:
            sp_c = np.broadcast_to(
                spat[None, :, :], (BL, U, F)
            ).reshape(NTILES, 128, F)
            m["sp"] = np.ascontiguousarray(sp_c)
        m["c0"] = np.ascontiguousarray(c0_full, np.float32)
        in_maps.append(m)

    res = bass_utils.run_bass_kernel_spmd(nc, in_maps, core_ids=list(range(NCORES)))
    out = np.empty((B, T, U), np.float32)
    for c in range(NCORES):
        yT = res.results[c]["y"].reshape(BL, U, T).astype(np.float32)
        out[c * BL : (c + 1) * BL] = np.swapaxes(yT, 1, 2)
    return out


if __name__ == "__main__":
    rng = np.random.default_rng(0)
    xs = rng.standard_normal((B, T, U)).astype(np.float32)
    e = np.exp(-0.001 / 0.1)
    sm = np.full((1, U), np.log(e / (1 - e)), np.float32)
    lv = np.zeros((1, U), np.float32)
    o = kernel(xs, lv, sm)
    print("out", o.shape, o.dtype, float(np.abs(o).max()))


# revision 5
# speedup vs baseline: 3.6775x; 1.3233x over previous
"""Trainium2 Bass kernel for nn_Lowpass: EMA recurrence over time.

y[b,t,u] = (1-s_u) x[b,t,u] + s_u y[b,t-1,u],   s = sigmoid(smoothing_var)

Strategy (scan formulation):
  - The DVE/Pool ISA op TensorTensorScanArith computes exactly this
    first-order recurrence along the free dimension with an fp32 internal
    state:  state = data0[:,t] * state + data1[:,t].
  - Layout: host transposes x to [batch, unit, time] so time is the free
    (contiguous) dim; partitions carry 128 units.  Per core (2 batches):
    16 tiles of [128 units, 2048 time], one scan op per tile - no carry
    chain, no matmuls, no PSUM.
  - data1 = (1-s)*x, pre-scaled on host (exact f32), stored fp16.
  - data0 = s must be a full fp16 [128,F] tensor.  fp16's grid near 0.99
    is ~4.9e-4 coarse (a 1.9% time-constant error), so data0 is a
    sigma-delta dither between the two fp16 grid neighbours of s chosen
    so every windowed product of data0 values tracks s^k to ~1e-3;
    the filter-shape error drops to ~0.05%.
  - CoreSim models every engine as one serial track that its DMA
    transfers fully occupy (~332 GB/s per engine-queue, queues on
    different engines overlap perfectly).  Scans only exist on DVE
    (2194 ns/tile) and Pool (1707 ns/tile).  Schedule: SP+ACT mostly DMA,
    Pool 9 scans, DVE 7 scans + a little DMA; outs spread by slack.
"""

import os
import sys
import functools

sys.path.insert(0, "/opt/trn_rl_repo")
os.environ.setdefault("MYCRO_LOCAL_CACHE", "1")

import numpy as np

B, T, U = 16, 2048, 1024
NCORES = 8
BL = B // NCORES            # batches per core
NTILES = BL * U // 128      # scan tiles per core
F = T                       # scan free length

# tiles scanned on DVE (rest on Pool); Pool also loads tiles 14,15
# (the real neuronxcc backend has no DVE DMA queue, so DVE only scans)
DVE_SCANS = (1, 3, 5, 7, 9, 11, 13, 15)
POOL_INS = (14, 15)
# out-DMA engine per tile
OUT_ENG = dict(zip(range(16), "sasasapasasasapp"))
ENG_ATTR = {"s": "sync", "a": "scalar", "d": "vector", "p": "gpsimd"}


@functools.lru_cache(maxsize=8)
def _build(uniform_s: bool, use_init: bool):
    import concourse.tile as tile
    from concourse import bacc, mybir

    nc = bacc.Bacc("TRN2", target_bir_lowering=False, debug=False)
    f32 = mybir.dt.float32
    f16 = mybir.dt.float16
    mult = mybir.AluOpType.mult
    add = mybir.AluOpType.add

    x = nc.dram_tensor("x", [NTILES, 128, F], f16, kind="ExternalInput").ap()
    sp = nc.dram_tensor(
        "sp", [1 if uniform_s else NTILES, 128, F], f16, kind="ExternalInput"
    ).ap()
    c0 = nc.dram_tensor("c0", [128, NTILES], f32, kind="ExternalInput").ap()
    y = nc.dram_tensor("y", [NTILES, 128, F], f16, kind="ExternalOutput").ap()

    # per-engine instruction streams; items ("s",) ("in",t) ("scan",t) ("out",t)
    sp_st = [("in", t) for t in range(0, NTILES, 2) if t not in POOL_INS]
    ac_st = [("in", t) for t in range(1, NTILES, 2) if t not in POOL_INS]
    dv_st = [("scan", t) for t in DVE_SCANS]
    pl_st = [("s",)] + [("in", t) for t in POOL_INS] + \
            [("scan", t) for t in range(NTILES) if t not in DVE_SCANS]
    streams = {"sync": sp_st, "scalar": ac_st, "vector": dv_st, "gpsimd": pl_st}
    for t in range(NTILES):
        streams[ENG_ATTR[OUT_ENG[t]]].append(("out", t))

    # interleave: emit round-robin, an item only after its producers
    order = []
    idx = {e: 0 for e in streams}
    done = set()
    while any(idx[e] < len(st) for e, st in streams.items()):
        progressed = False
        for e, st in streams.items():
            if idx[e] >= len(st):
                continue
            item = st[idx[e]]
            kind = item[0]
            ready = (
                kind in ("in", "s")
                or (kind == "scan" and ("in", item[1]) in done
                    and (uniform_s and ("s",) in done or not uniform_s))
                or (kind == "out" and ("scan", item[1]) in done)
            )
            if ready:
                order.append((e, item))
                done.add(item)
                idx[e] += 1
                progressed = True
        assert progressed, "deadlock in stream emission"

    with tile.TileContext(nc) as tc:
        with (
            tc.tile_pool(
                name="const",
                bufs=(1 if uniform_s else NTILES) + (1 if use_init else 0),
            ) as constp,
            tc.tile_pool(name="xin", bufs=16) as xinp,
            tc.tile_pool(name="yout", bufs=16) as youtp,
        ):
            if not uniform_s:
                stiles = []
                qrr = ["sync", "scalar", "vector", "gpsimd"]
                for i in range(NTILES):
                    sti = constp.tile([128, F], f16)
                    getattr(nc, qrr[i % 4]).dma_start(sti[:, :], sp[i])
                    stiles.append(sti)
            if use_init:
                c0t = constp.tile([128, NTILES], f32)
                nc.scalar.dma_start(c0t[:, :], c0)

            st_tile = None
            xts, yts = {}, {}
            for eng, item in order:
                kind = item[0]
                if kind == "s":
                    if uniform_s:
                        st_tile = constp.tile([128, F], f16)
                        getattr(nc, eng).dma_start(st_tile[:, :], sp[0])
                elif kind == "in":
                    t = item[1]
                    xt = xinp.tile([128, F], f16)
                    getattr(nc, eng).dma_start(xt[:, :], x[t])
                    xts[t] = xt
                elif kind == "scan":
                    t = item[1]
                    yt = youtp.tile([128, F], f16)
                    d0 = st_tile if uniform_s else stiles[t]
                    init = c0t[:, t : t + 1] if use_init else 0.0
                    getattr(nc, eng).tensor_tensor_scan(
                        out=yt[:, :], data0=d0[:, :], data1=xts[t][:, :],
                        initial=init, op0=mult, op1=add)
                    yts[t] = yt
                elif kind == "out":
                    t = item[1]
                    getattr(nc, eng).dma_start(y[t], yts[t][:, :])
    nc.compile()
    return nc


def _fp16_neighbors(s64):
    a = np.float16(s64)
    fa = float(np.float64(a))
    if fa == s64:
        return a, a
    if fa < s64:
        b = np.nextafter(a, np.float16(np.inf))
    else:
        b = np.nextafter(a, np.float16(-np.inf))
    return a, b


def _dither_pattern(s64, n):
    """fp16 sequence whose windowed products track s64^k to ~one grid step."""
    va, vb = _fp16_neighbors(s64)
    if va == vb:
        return np.full(n, va, np.float16)
    la = float(np.log(np.float64(va)))
    lb = float(np.log(np.float64(vb)))
    tgt = float(np.log(np.float64(s64)))
    out = np.empty(n, np.float16)
    cum = 0.0
    for t in range(n):
        ea = cum + la - tgt
        eb = cum + lb - tgt
        if abs(ea) <= abs(eb):
            out[t] = va
            cum = ea
        else:
            out[t] = vb
            cum = eb
    return out


def _prep_host(inputs, level_var, smoothing_var):
    x = np.asarray(inputs, np.float32)
    assert x.shape == (B, T, U), x.shape
    sm = np.asarray(smoothing_var, np.float32).reshape(-1)
    s64 = 1.0 / (1.0 + np.exp(-sm.astype(np.float64)))
    s32 = s64.astype(np.float32)

    xs = x * (np.float32(1.0) - s32)[None, None, :]
    xsT = np.ascontiguousarray(np.swapaxes(xs, 1, 2)).astype(np.float16)

    uniform = bool(np.all(s64 == s64[0]))
    if uniform:
        pat = _dither_pattern(float(s64[0]), F)
        spat = np.ascontiguousarray(
            np.broadcast_to(pat[None, None, :], (1, 128, F))
        )
    else:
        spat = np.empty((U, F), np.float16)
        seen = {}
        for u in range(U):
            k = float(s64[u])
            if k not in seen:
                seen[k] = _dither_pattern(k, F)
            spat[u] = seen[k]

    lv = np.asarray(level_var, np.float32).reshape(-1)
    use_init = bool(np.any(lv != 0.0))
    return xsT, spat, s32, lv, uniform, use_init


def _in_map(xsT, spat, lv, uniform, core):
    xc = xsT[core * BL : (core + 1) * BL].reshape(NTILES, 128, F)
    m = {"x": np.ascontiguousarray(xc)}
    if uniform:
        m["sp"] = spat
    else:
        sp_c = np.broadcast_to(spat[None, :, :], (BL, U, F)).reshape(
            NTILES, 128, F
        )
        m["sp"] = np.ascontiguousarray(sp_c)
    c0 = np.broadcast_to(lv[None, :], (BL, U)).reshape(NTILES, 128)
    m["c0"] = np.ascontiguousarray(c0.T, np.float32).reshape(128, NTILES)
    return m


def kernel(inputs: np.ndarray, level_var: np.ndarray, smoothing_var: np.ndarray):
    from concourse import bass_utils

    xsT, spat, s32, lv, uniform, use_init = _prep_host(
        inputs, level_var, smoothing_var
    )
    nc = _build(uniform, use_init)
    in_maps = [_in_map(xsT, spat, lv, uniform, c) for c in range(NCORES)]
    res = bass_utils.run_bass_kernel_spmd(nc, in_maps, core_ids=list(range(NCORES)))
    out = np.empty((B, T, U), np.float32)
    for c in range(NCORES):
        yT = res.results[c]["y"].reshape(BL, U, T).astype(np.float32)
        out[c * BL : (c + 1) * BL] = np.swapaxes(yT, 1, 2)
    return out


if __name__ == "__main__":
    rng = np.random.default_rng(0)
    xs = rng.standard_normal((B, T, U)).astype(np.float32)
    e = np.exp(-0.001 / 0.1)
    sm = np.full((1, U), np.log(e / (1 - e)), np.float32)
    lv = np.zeros((1, U), np.float32)
    o = kernel(xs, lv, sm)
    print("out", o.shape, o.dtype, float(np.abs(o).max()))
